# revision 1
# baseline (speedup 1.0000x reference)
"""Cross-attention Trainium2 kernel (Bass/Tile), data-parallel over batch.

B=8 batch elements -> 8 NeuronCores, one batch element per core.
Per core: y = softmax(q Wq (kv Wk)^T / sqrt(dk)) (kv Wv) Wo + bo
with S1=S2=2048, D=1024, H=8, DK=DV=128.

Layout strategy (everything bf16 on the PE, fp32 softmax stats):
  - inputs are cast fp32->bf16 during the SWDGE load, then DMA-xbar-transposed
    to qT/kvT [D, S] tiles.
  - projections produce QT,KT  [H*DK, S] (head-major partition chunks) and
    V [S2, H*DV] (natural), all bf16 in SBUF.
  - scores S = QT_h^T KT_h computed per 128-row q-block into PSUM, exp on ACT
    with fused accum_out row-sums (no max subtraction: |s*scale| < ~3),
    normalize P on DVE, DMA-xbar-transpose P -> PT, then O^T = sum_c V_c^T PT_c
    so the output projection can consume O^T directly with Wo natural.
  - bias bo is folded in as a K=1 ones x bo matmul that opens each output
    accumulation group.
"""

import os

import numpy as np

import concourse.bass as bass
import concourse.mybir as mybir
import concourse.tile as tile
from concourse import bacc
from concourse.bass_utils import run_bass_kernel_spmd

B = 8
S = 2048  # S1 == S2
D = 1024  # D1 == D2
H = 8
DK = DV = 128
KC = D // 128  # contraction chunks
SC = S // 128  # sequence chunks of 128
BLK = 512
NBLK = S // BLK
SCALE = 1.0 / float(np.sqrt(DK))

F32 = mybir.dt.float32
BF16 = mybir.dt.bfloat16
EXP = mybir.ActivationFunctionType.Exp


def _emit(tc, aps):
    nc = tc.nc
    query, key_value, Wq, Wk, Wv, Wo, bo, out = (
        aps["query"], aps["key_value"], aps["Wq"], aps["Wk"], aps["Wv"],
        aps["Wo"], aps["bo"], aps["out"],
    )

    persist = tc.alloc_tile_pool(name="persist", bufs=1)
    QT_sb = persist.tile([128, H, S], BF16, name="QT_sb")
    KT_sb = persist.tile([128, H, S], BF16, name="KT_sb")
    V_sb = persist.tile([128, SC, H * DV], BF16, name="V_sb")
    Wo_sb = persist.tile([128, KC, D], BF16, name="Wo_sb")
    bo_sb = persist.tile([1, D], BF16, name="bo_sb")
    ones_sb = persist.tile([1, 128], BF16, name="ones_sb")

    nc.vector.memset(ones_sb, 1.0)
    nc.gpsimd.dma_start(out=bo_sb, in_=bo)  # casts f32 -> bf16
    nc.gpsimd.dma_start(
        out=Wo_sb, in_=Wo.rearrange("(kc p) n -> p kc n", p=128)
    )

    # ---- phase 1: projections ----------------------------------------
    def load_transposed_block(work, src_ap, j, tag):
        """Load 512 rows of src [S, D] f32, return xT block [128, KC, 512] bf16."""
        xT = work.tile([128, KC, BLK], BF16, name=f"{tag}T", tag=f"{tag}T", bufs=2)
        for c4 in range(4):
            c = j * 4 + c4
            row = work.tile([128, D], BF16, name=f"{tag}row", tag="row", bufs=3)
            nc.gpsimd.dma_start(out=row, in_=src_ap[c * 128:(c + 1) * 128, :])
            for kc in range(KC):
                nc.sync.dma_start(
                    out=xT[:, kc, c4 * 128:(c4 + 1) * 128],
                    in_=row[:, kc * 128:(kc + 1) * 128],
                    transpose=True,
                )
        return xT

    with tc.tile_pool(name="p1w_kv", bufs=1) as wkv, \
         tc.tile_pool(name="p1work_kv", bufs=1) as work, \
         tc.tile_pool(name="p1psum_kv", bufs=4, space="PSUM") as pps:
        Wk_sb = wkv.tile([128, KC, D], BF16, name="Wk_sb")
        Wv_sb = wkv.tile([128, KC, D], BF16, name="Wv_sb")
        nc.gpsimd.dma_start(out=Wk_sb, in_=Wk.rearrange("(kc p) n -> p kc n", p=128))
        nc.gpsimd.dma_start(out=Wv_sb, in_=Wv.rearrange("(kc p) n -> p kc n", p=128))
        for j in range(NBLK):
            kvT = load_transposed_block(work, key_value, j, "kv")
            # KT block: out[M=dk chunk m (head), N=s2] += Wk[kc,m].T @ kvT[kc]
            for m in range(H):
                ps = pps.tile([128, BLK], F32, name="ps_k", tag="pps")
                for kc in range(KC):
                    nc.tensor.matmul(
                        ps, lhsT=Wk_sb[:, kc, m * 128:(m + 1) * 128],
                        rhs=kvT[:, kc, :], start=(kc == 0), stop=(kc == KC - 1),
                    )
                nc.scalar.copy(KT_sb[:, m, j * BLK:(j + 1) * BLK], ps)
            # V block rows: out[M=s2 sub, N=hdv] += kvT[kc, sub].T @ Wv[kc]
            for m4 in range(4):
                for n in range(2):
                    ps = pps.tile([128, BLK], F32, name="ps_v", tag="pps")
                    for kc in range(KC):
                        nc.tensor.matmul(
                            ps, lhsT=kvT[:, kc, m4 * 128:(m4 + 1) * 128],
                            rhs=Wv_sb[:, kc, n * BLK:(n + 1) * BLK],
                            start=(kc == 0), stop=(kc == KC - 1),
                        )
                    nc.scalar.copy(
                        V_sb[:, j * 4 + m4, n * BLK:(n + 1) * BLK], ps
                    )

    with tc.tile_pool(name="p1w_q", bufs=1) as wq, \
         tc.tile_pool(name="p1work_q", bufs=1) as work, \
         tc.tile_pool(name="p1psum_q", bufs=4, space="PSUM") as pps:
        Wq_sb = wq.tile([128, KC, D], BF16, name="Wq_sb")
        nc.gpsimd.dma_start(out=Wq_sb, in_=Wq.rearrange("(kc p) n -> p kc n", p=128))
        for j in range(NBLK):
            qT = load_transposed_block(work, query, j, "q")
            for m in range(H):
                ps = pps.tile([128, BLK], F32, name="ps_q", tag="pps")
                for kc in range(KC):
                    nc.tensor.matmul(
                        ps, lhsT=Wq_sb[:, kc, m * 128:(m + 1) * 128],
                        rhs=qT[:, kc, :], start=(kc == 0), stop=(kc == KC - 1),
                    )
                nc.scalar.copy(QT_sb[:, m, j * BLK:(j + 1) * BLK], ps)

    # ---- phase 2+3: attention + output projection --------------------
    with tc.tile_pool(name="p2", bufs=1) as p2, \
         tc.tile_pool(name="small", bufs=1) as small, \
         tc.tile_pool(name="spsum", bufs=2, space="PSUM") as spsum, \
         tc.tile_pool(name="opsum", bufs=2, space="PSUM") as opsum, \
         tc.tile_pool(name="ypsum", bufs=2, space="PSUM") as ypsum:
        for j in range(NBLK):
            OT_sb = p2.tile([128, H, BLK], BF16, name="OT_sb", tag="OT", bufs=2)
            for h in range(H):
                PT_sb = p2.tile([128, SC, BLK], BF16, name="PT_sb", tag="PT", bufs=2)
                for sub in range(4):
                    qcol = j * BLK + sub * 128
                    qtile = QT_sb[:, h, qcol:qcol + 128]
                    P_sb = p2.tile([128, S], BF16, name="P_sb", tag="P", bufs=3)
                    ssum = small.tile([128, 1], F32, name="ssum", tag="ssum", bufs=8)
                    rec = small.tile([128, 1], F32, name="rec", tag="rec", bufs=8)
                    for half in range(2):
                        sps = spsum.tile([128, 1024], F32, name="sps", tag="sps")
                        for n in range(2):
                            nc.tensor.matmul(
                                sps[:, n * BLK:(n + 1) * BLK],
                                lhsT=qtile,
                                rhs=KT_sb[:, h, half * 1024 + n * BLK:
                                          half * 1024 + (n + 1) * BLK],
                                start=True, stop=True,
                            )
                        acc = small.tile([128, 1], F32, name="acc", tag=f"acc{half}",
                                         bufs=4)
                        nc.scalar.activation(
                            P_sb[:, half * 1024:(half + 1) * 1024], sps, EXP,
                            scale=SCALE, accum_out=acc,
                        )
                        if half == 0:
                            acc0 = acc
                    nc.vector.tensor_add(ssum, acc0, acc)
                    nc.vector.reciprocal(rec, ssum)
                    nc.vector.tensor_scalar_mul(P_sb, P_sb, rec)
                    for c in range(SC):
                        nc.sync.dma_start(
                            out=PT_sb[:, c, sub * 128:(sub + 1) * 128],
                            in_=P_sb[:, c * 128:(c + 1) * 128],
                            transpose=True,
                        )
                ops = opsum.tile([128, BLK], F32, name="ops", tag="ops")
                for c in range(SC):
                    nc.tensor.matmul(
                        ops, lhsT=V_sb[:, c, h * 128:(h + 1) * 128],
                        rhs=PT_sb[:, c, :], start=(c == 0), stop=(c == SC - 1),
                    )
                nc.vector.tensor_copy(OT_sb[:, h, :], ops)
            # output projection for block j
            for m in range(4):
                for n in range(2):
                    yps = ypsum.tile([128, BLK], F32, name="yps", tag="yps")
                    nc.tensor.matmul(
                        yps, lhsT=ones_sb, rhs=bo_sb[:, n * BLK:(n + 1) * BLK],
                        start=True, stop=False,
                    )
                    for h in range(H):
                        nc.tensor.matmul(
                            yps, lhsT=OT_sb[:, h, m * 128:(m + 1) * 128],
                            rhs=Wo_sb[:, h, n * BLK:(n + 1) * BLK],
                            start=False, stop=(h == H - 1),
                        )
                    y_sb = p2.tile([128, BLK], F32, name="y_sb", tag="y", bufs=3)
                    nc.scalar.copy(y_sb, yps)
                    r0 = j * BLK + m * 128
                    nc.sync.dma_start(
                        out=out[r0:r0 + 128, n * BLK:(n + 1) * BLK], in_=y_sb
                    )
    persist.release()


_CACHE = {}


def _build():
    if "nc" in _CACHE:
        return _CACHE["nc"]
    nc = bacc.Bacc(
        "TRN2", target_bir_lowering=False, debug=False,
        enable_asserts=False, num_devices=B,
    )
    aps = {
        "query": nc.dram_tensor("query", [S, D], F32, kind="ExternalInput").ap(),
        "key_value": nc.dram_tensor("key_value", [S, D], F32, kind="ExternalInput").ap(),
        "Wq": nc.dram_tensor("Wq", [D, H * DK], F32, kind="ExternalInput").ap(),
        "Wk": nc.dram_tensor("Wk", [D, H * DK], F32, kind="ExternalInput").ap(),
        "Wv": nc.dram_tensor("Wv", [D, H * DV], F32, kind="ExternalInput").ap(),
        "Wo": nc.dram_tensor("Wo", [H * DV, D], F32, kind="ExternalInput").ap(),
        "bo": nc.dram_tensor("bo", [1, D], F32, kind="ExternalInput").ap(),
        "out": nc.dram_tensor("out", [S, D], F32, kind="ExternalOutput").ap(),
    }
    with tile.TileContext(nc) as tc:
        _emit(tc, aps)
    nc.compile()
    _CACHE["nc"] = nc
    return nc


LAST_RESULT = None


def kernel(query, key_value, Wq, Wk, Wv, Wo, bo):
    global LAST_RESULT
    nc = _build()
    query = np.ascontiguousarray(np.asarray(query, dtype=np.float32))
    key_value = np.ascontiguousarray(np.asarray(key_value, dtype=np.float32))
    shared = {
        "Wq": np.ascontiguousarray(np.asarray(Wq, dtype=np.float32)),
        "Wk": np.ascontiguousarray(np.asarray(Wk, dtype=np.float32)),
        "Wv": np.ascontiguousarray(np.asarray(Wv, dtype=np.float32)),
        "Wo": np.ascontiguousarray(np.asarray(Wo, dtype=np.float32)),
        "bo": np.ascontiguousarray(np.asarray(bo, dtype=np.float32)).reshape(1, D),
    }
    in_maps = [
        {"query": query[i], "key_value": key_value[i], **shared} for i in range(B)
    ]
    res = run_bass_kernel_spmd(
        nc, in_maps, core_ids=list(range(B)),
        trace=bool(int(os.environ.get("KERNEL_TRACE", "0"))),
    )
    LAST_RESULT = res
    return np.stack([r["out"] for r in res.results]).astype(np.float32)


if __name__ == "__main__":
    rng = np.random.default_rng(0)
    inputs = {
        "query": rng.standard_normal((B, S, D), dtype=np.float32),
        "key_value": rng.standard_normal((B, S, D), dtype=np.float32),
        "Wq": (rng.random((D, H * DK), dtype=np.float32) - 0.5) / 16.0,
        "Wk": (rng.random((D, H * DK), dtype=np.float32) - 0.5) / 16.0,
        "Wv": (rng.random((D, H * DV), dtype=np.float32) - 0.5) / 16.0,
        "Wo": (rng.random((H * DV, D), dtype=np.float32) - 0.5) / 16.0,
        "bo": (rng.random(D, dtype=np.float32) - 0.5) / 16.0,
    }
    y = kernel(**inputs)
    print("kernel out", y.shape, y.dtype, float(np.abs(y).max()))


# revision 4
# speedup vs baseline: 1709.6112x; 1709.6112x over previous
"""Cross-attention Trainium2 kernel (Bass/Tile), data-parallel over batch.

B=8 batch elements -> 8 NeuronCores, one batch element per core.
Per core: y = softmax(q Wq (kv Wk)^T / sqrt(dk)) (kv Wv) Wo + bo
with S1=S2=2048, D=1024, H=8, DK=DV=128.

Layout strategy (everything bf16 on the PE, fp32 softmax stats):
  - inputs are cast fp32->bf16 during the SWDGE load, then DMA-xbar-transposed
    to qT/kvT [D, S] tiles.
  - projections produce QT,KT  [H*DK, S] (head-major partition chunks) and
    V [S2, H*DV] (natural), all bf16 in SBUF.
  - scores S = QT_h^T KT_h computed per 128-row q-block into PSUM, exp on ACT
    with fused accum_out row-sums (no max subtraction: |s*scale| < ~3),
    normalize P on DVE, DMA-xbar-transpose P -> PT, then O^T = sum_c V_c^T PT_c
    so the output projection can consume O^T directly with Wo natural.
  - bias bo is folded in as a K=1 ones x bo matmul that opens each output
    accumulation group.
"""

import os

import numpy as np

import concourse.bass as bass
import concourse.mybir as mybir
import concourse.tile as tile
from concourse import bacc
from concourse.bass_utils import run_bass_kernel_spmd

B = 8
S = 2048  # S1 == S2
D = 1024  # D1 == D2
H = 8
DK = DV = 128
KC = D // 128  # contraction chunks
SC = S // 128  # sequence chunks of 128
BLK = 512
NBLK = S // BLK
SCALE = 1.0 / float(np.sqrt(DK))

F32 = mybir.dt.float32
BF16 = mybir.dt.bfloat16
EXP = mybir.ActivationFunctionType.Exp


def _emit(tc, aps):
    nc = tc.nc
    query, key_value, Wq, Wk, Wv, Wo, bo, out = (
        aps["query"], aps["key_value"], aps["Wq"], aps["Wk"], aps["Wv"],
        aps["Wo"], aps["bo"], aps["out"],
    )

    persist = tc.alloc_tile_pool(name="persist", bufs=1)
    QT_sb = persist.tile([128, H, S], BF16, name="QT_sb")
    KT_sb = persist.tile([128, H, S], BF16, name="KT_sb")
    V_sb = persist.tile([128, SC, H * DV], BF16, name="V_sb")
    Wo_sb = persist.tile([128, KC, D], BF16, name="Wo_sb")
    bo_sb = persist.tile([1, D], BF16, name="bo_sb")
    ones_sb = persist.tile([1, 128], BF16, name="ones_sb")

    nc.vector.memset(ones_sb, 1.0)
    nc.gpsimd.dma_start(out=bo_sb, in_=bo)  # casts f32 -> bf16
    nc.gpsimd.dma_start(
        out=Wo_sb, in_=Wo.rearrange("(kc p) n -> p kc n", p=128)
    )

    # ---- phase 1: projections ----------------------------------------
    def load_transposed_block(work, src_ap, j, tag):
        """Load 512 rows of src [S, D] f32, return xT block [128, KC, 512] bf16."""
        xT = work.tile([128, KC, BLK], BF16, name=f"{tag}T", tag=f"{tag}T", bufs=2)
        for c4 in range(4):
            c = j * 4 + c4
            row = work.tile([128, D], BF16, name=f"{tag}row", tag="row", bufs=3)
            nc.gpsimd.dma_start(out=row, in_=src_ap[c * 128:(c + 1) * 128, :])
            for kc in range(KC):
                nc.sync.dma_start(
                    out=xT[:, kc, c4 * 128:(c4 + 1) * 128],
                    in_=row[:, kc * 128:(kc + 1) * 128],
                    transpose=True,
                )
        return xT

    with nc.named_scope("ph1_kv"), \
         tc.tile_pool(name="p1w_kv", bufs=1) as wkv, \
         tc.tile_pool(name="p1work_kv", bufs=1) as work, \
         tc.tile_pool(name="p1psum_kv", bufs=4, space="PSUM") as pps:
        Wk_sb = wkv.tile([128, KC, D], BF16, name="Wk_sb")
        Wv_sb = wkv.tile([128, KC, D], BF16, name="Wv_sb")
        nc.gpsimd.dma_start(out=Wk_sb, in_=Wk.rearrange("(kc p) n -> p kc n", p=128))
        nc.gpsimd.dma_start(out=Wv_sb, in_=Wv.rearrange("(kc p) n -> p kc n", p=128))
        for j in range(NBLK):
            kvT = load_transposed_block(work, key_value, j, "kv")
            # KT block: out[M=dk chunk m (head), N=s2] += Wk[kc,m].T @ kvT[kc]
            for m in range(H):
                ps = pps.tile([128, BLK], F32, name="ps_k", tag="pps")
                for kc in range(KC):
                    nc.tensor.matmul(
                        ps, lhsT=Wk_sb[:, kc, m * 128:(m + 1) * 128],
                        rhs=kvT[:, kc, :], start=(kc == 0), stop=(kc == KC - 1),
                    )
                nc.scalar.copy(KT_sb[:, m, j * BLK:(j + 1) * BLK], ps)
            # V block rows: out[M=s2 sub, N=hdv] += kvT[kc, sub].T @ Wv[kc]
            for m4 in range(4):
                for n in range(2):
                    ps = pps.tile([128, BLK], F32, name="ps_v", tag="pps")
                    for kc in range(KC):
                        nc.tensor.matmul(
                            ps, lhsT=kvT[:, kc, m4 * 128:(m4 + 1) * 128],
                            rhs=Wv_sb[:, kc, n * BLK:(n + 1) * BLK],
                            start=(kc == 0), stop=(kc == KC - 1),
                        )
                    nc.scalar.copy(
                        V_sb[:, j * 4 + m4, n * BLK:(n + 1) * BLK], ps
                    )

    with nc.named_scope("ph1_q"), \
         tc.tile_pool(name="p1w_q", bufs=1) as wq, \
         tc.tile_pool(name="p1work_q", bufs=1) as work, \
         tc.tile_pool(name="p1psum_q", bufs=4, space="PSUM") as pps:
        Wq_sb = wq.tile([128, KC, D], BF16, name="Wq_sb")
        nc.gpsimd.dma_start(out=Wq_sb, in_=Wq.rearrange("(kc p) n -> p kc n", p=128))
        for j in range(NBLK):
            qT = load_transposed_block(work, query, j, "q")
            for m in range(H):
                ps = pps.tile([128, BLK], F32, name="ps_q", tag="pps")
                for kc in range(KC):
                    nc.tensor.matmul(
                        ps, lhsT=Wq_sb[:, kc, m * 128:(m + 1) * 128],
                        rhs=qT[:, kc, :], start=(kc == 0), stop=(kc == KC - 1),
                    )
                nc.scalar.copy(QT_sb[:, m, j * BLK:(j + 1) * BLK], ps)

    # ---- phase 2+3: attention + output projection --------------------
    with nc.named_scope("attn"), \
         tc.tile_pool(name="p2", bufs=1) as p2, \
         tc.tile_pool(name="small", bufs=1) as small, \
         tc.tile_pool(name="spsum", bufs=2, space="PSUM") as spsum, \
         tc.tile_pool(name="opsum", bufs=2, space="PSUM") as opsum, \
         tc.tile_pool(name="ypsum", bufs=2, space="PSUM") as ypsum:
        for j in range(NBLK):
            OT_sb = p2.tile([128, H, BLK], BF16, name="OT_sb", tag="OT", bufs=2)
            for h in range(H):
                PT_sb = p2.tile([128, SC, BLK], BF16, name="PT_sb", tag="PT", bufs=2)
                for sub in range(4):
                    qcol = j * BLK + sub * 128
                    qtile = QT_sb[:, h, qcol:qcol + 128]
                    P_sb = p2.tile([128, S], BF16, name="P_sb", tag="P", bufs=3)
                    ssum = small.tile([128, 1], F32, name="ssum", tag="ssum", bufs=8)
                    rec = small.tile([128, 1], F32, name="rec", tag="rec", bufs=8)
                    for half in range(2):
                        sps = spsum.tile([128, 1024], F32, name="sps", tag="sps")
                        for n in range(2):
                            nc.tensor.matmul(
                                sps[:, n * BLK:(n + 1) * BLK],
                                lhsT=qtile,
                                rhs=KT_sb[:, h, half * 1024 + n * BLK:
                                          half * 1024 + (n + 1) * BLK],
                                start=True, stop=True,
                            )
                        acc = small.tile([128, 1], F32, name="acc", tag=f"acc{half}",
                                         bufs=4)
                        nc.scalar.activation(
                            P_sb[:, half * 1024:(half + 1) * 1024], sps, EXP,
                            scale=SCALE, accum_out=acc,
                        )
                        if half == 0:
                            acc0 = acc
                    nc.vector.tensor_add(ssum, acc0, acc)
                    nc.vector.reciprocal(rec, ssum)
                    nc.vector.tensor_scalar_mul(P_sb, P_sb, rec)
                    for c in range(SC):
                        nc.sync.dma_start(
                            out=PT_sb[:, c, sub * 128:(sub + 1) * 128],
                            in_=P_sb[:, c * 128:(c + 1) * 128],
                            transpose=True,
                        )
                ops = opsum.tile([128, BLK], F32, name="ops", tag="ops")
                for c in range(SC):
                    nc.tensor.matmul(
                        ops, lhsT=V_sb[:, c, h * 128:(h + 1) * 128],
                        rhs=PT_sb[:, c, :], start=(c == 0), stop=(c == SC - 1),
                    )
                nc.vector.tensor_copy(OT_sb[:, h, :], ops)
            # output projection for block j
            for m in range(4):
                for n in range(2):
                    yps = ypsum.tile([128, BLK], F32, name="yps", tag="yps")
                    nc.tensor.matmul(
                        yps, lhsT=ones_sb, rhs=bo_sb[:, n * BLK:(n + 1) * BLK],
                        start=True, stop=False,
                    )
                    for h in range(H):
                        nc.tensor.matmul(
                            yps, lhsT=OT_sb[:, h, m * 128:(m + 1) * 128],
                            rhs=Wo_sb[:, h, n * BLK:(n + 1) * BLK],
                            start=False, stop=(h == H - 1),
                        )
                    y_sb = p2.tile([128, BLK], F32, name="y_sb", tag="y", bufs=3)
                    nc.scalar.copy(y_sb, yps)
                    r0 = j * BLK + m * 128
                    nc.sync.dma_start(
                        out=out[r0:r0 + 128, n * BLK:(n + 1) * BLK], in_=y_sb
                    )
    persist.release()


_CACHE = {}


def _build():
    if "nc" in _CACHE:
        return _CACHE["nc"]
    nc = bacc.Bacc(
        "TRN2", target_bir_lowering=False, debug=False,
        enable_asserts=False, num_devices=B,
    )
    aps = {
        "query": nc.dram_tensor("query", [S, D], F32, kind="ExternalInput").ap(),
        "key_value": nc.dram_tensor("key_value", [S, D], F32, kind="ExternalInput").ap(),
        "Wq": nc.dram_tensor("Wq", [D, H * DK], F32, kind="ExternalInput").ap(),
        "Wk": nc.dram_tensor("Wk", [D, H * DK], F32, kind="ExternalInput").ap(),
        "Wv": nc.dram_tensor("Wv", [D, H * DV], F32, kind="ExternalInput").ap(),
        "Wo": nc.dram_tensor("Wo", [H * DV, D], F32, kind="ExternalInput").ap(),
        "bo": nc.dram_tensor("bo", [1, D], F32, kind="ExternalInput").ap(),
        "out": nc.dram_tensor("out", [S, D], F32, kind="ExternalOutput").ap(),
    }
    with tile.TileContext(nc) as tc:
        _emit(tc, aps)
    nc.compile()
    _CACHE["nc"] = nc
    return nc


LAST_RESULT = None


def kernel(query, key_value, Wq, Wk, Wv, Wo, bo):
    global LAST_RESULT
    nc = _build()
    query = np.ascontiguousarray(np.asarray(query, dtype=np.float32))
    key_value = np.ascontiguousarray(np.asarray(key_value, dtype=np.float32))
    shared = {
        "Wq": np.ascontiguousarray(np.asarray(Wq, dtype=np.float32)),
        "Wk": np.ascontiguousarray(np.asarray(Wk, dtype=np.float32)),
        "Wv": np.ascontiguousarray(np.asarray(Wv, dtype=np.float32)),
        "Wo": np.ascontiguousarray(np.asarray(Wo, dtype=np.float32)),
        "bo": np.ascontiguousarray(np.asarray(bo, dtype=np.float32)).reshape(1, D),
    }
    in_maps = [
        {"query": query[i], "key_value": key_value[i], **shared} for i in range(B)
    ]
    res = run_bass_kernel_spmd(
        nc, in_maps, core_ids=list(range(B)),
        trace=bool(int(os.environ.get("KERNEL_TRACE", "0"))),
    )
    LAST_RESULT = res
    return np.stack([r["out"] for r in res.results]).astype(np.float32)


if __name__ == "__main__":
    rng = np.random.default_rng(0)
    inputs = {
        "query": rng.standard_normal((B, S, D), dtype=np.float32),
        "key_value": rng.standard_normal((B, S, D), dtype=np.float32),
        "Wq": (rng.random((D, H * DK), dtype=np.float32) - 0.5) / 16.0,
        "Wk": (rng.random((D, H * DK), dtype=np.float32) - 0.5) / 16.0,
        "Wv": (rng.random((D, H * DV), dtype=np.float32) - 0.5) / 16.0,
        "Wo": (rng.random((H * DV, D), dtype=np.float32) - 0.5) / 16.0,
        "bo": (rng.random(D, dtype=np.float32) - 0.5) / 16.0,
    }
    y = kernel(**inputs)
    print("kernel out", y.shape, y.dtype, float(np.abs(y).max()))


# revision 14
# speedup vs baseline: 6120.9691x; 3.5803x over previous
"""Cross-attention Trainium2 kernel (Bass/Tile), data-parallel over batch.

B=8 batch elements -> 8 NeuronCores, one batch element per core.
Per core: y = softmax(q Wq (kv Wk)^T / sqrt(dk)) (kv Wv) Wo + bo
with S1=S2=2048, D=1024, H=8, DK=DV=128.

Layout strategy (everything bf16 on the PE, fp32 softmax stats):
  - inputs are cast fp32->bf16 during the SWDGE load, then DMA-xbar-transposed
    to qT/kvT [D, S] tiles.
  - projections produce QT,KT  [H*DK, S] (head-major partition chunks) and
    V [S2, H*DV] (natural), all bf16 in SBUF.
  - scores S = QT_h^T KT_h computed per 128-row q-block into PSUM, exp on ACT
    with fused accum_out row-sums (no max subtraction: |s*scale| < ~3),
    normalize P on DVE, DMA-xbar-transpose P -> PT, then O^T = sum_c V_c^T PT_c
    so the output projection can consume O^T directly with Wo natural.
  - bias bo is folded in as a K=1 ones x bo matmul that opens each output
    accumulation group.
"""

import os

import numpy as np

import concourse.bass as bass
import concourse.mybir as mybir
import concourse.tile as tile
from concourse import bacc
from concourse.bass_utils import run_bass_kernel_spmd
from concourse.masks import make_identity

B = 8
S = 2048  # S1 == S2
D = 1024  # D1 == D2
H = 8
DK = DV = 128
KC = D // 128  # contraction chunks
SC = S // 128  # sequence chunks of 128
BLK = 512
NBLK = S // BLK
SCALE = 1.0 / float(np.sqrt(DK))

F32 = mybir.dt.float32
BF16 = mybir.dt.bfloat16
EXP = mybir.ActivationFunctionType.Exp


def _emit(tc, aps):
    nc = tc.nc
    query, key_value, Wq, Wk, Wv, Wo, bo, out = (
        aps["query"], aps["key_value"], aps["Wq"], aps["Wk"], aps["Wv"],
        aps["Wo"], aps["bo"], aps["out"],
    )

    persist = tc.alloc_tile_pool(name="persist", bufs=1)
    QT_sb = persist.tile([128, H, S], BF16, name="QT_sb")
    KT_sb = persist.tile([128, H, S], BF16, name="KT_sb")
    V_sb = persist.tile([128, SC, H * DV], BF16, name="V_sb")
    Wo_sb = persist.tile([128, KC, D], BF16, name="Wo_sb")
    bo_sb = persist.tile([1, D], BF16, name="bo_sb")
    ones_sb = persist.tile([1, 128], BF16, name="ones_sb")

    ident = persist.tile([128, 128], BF16, name="ident")
    make_identity(nc, ident)
    nc.vector.memset(ones_sb, 1.0)
    nc.gpsimd.dma_start(out=bo_sb, in_=bo)  # casts f32 -> bf16
    nc.gpsimd.dma_start(
        out=Wo_sb, in_=Wo.rearrange("(kc p) n -> p kc n", p=128)
    )

    def pe_transpose8(tpool, dst8, src, cols, copy_engine):
        """Transpose eight [128,128] bf16 tiles of src (cols slice list) through
        one PSUM bank and copy into dst8 [128, 8, 128]."""
        tp = tpool.tile([128, 1024], BF16, name="tp", tag="tp")
        for i, c0 in enumerate(cols):
            nc.tensor.transpose(
                tp[:, i * 128:(i + 1) * 128], src[:, c0:c0 + 128], ident
            )
        srcv = tp.rearrange("p (c f) -> p c f", c=8)
        if copy_engine == 0:
            nc.vector.tensor_copy(dst8, srcv)
        else:
            nc.scalar.copy(dst8, srcv)

    # ---- phase 1: projections ----------------------------------------
    def load_transposed_block(work, tpool, src_ap, j, tag):
        """Load 512 rows of src [S, D] f32, return xT block [128, KC, 512] bf16."""
        xT = work.tile([128, KC, BLK], BF16, name=f"{tag}T", tag=f"{tag}T", bufs=2)
        for c4 in range(4):
            c = j * 4 + c4
            row = work.tile([128, D], BF16, name=f"{tag}row", tag="row", bufs=3)
            nc.gpsimd.dma_start(out=row, in_=src_ap[c * 128:(c + 1) * 128, :])
            pe_transpose8(
                tpool,
                xT[:, :, c4 * 128:(c4 + 1) * 128],
                row, [kc * 128 for kc in range(KC)],
                copy_engine=c4 % 2,
            )
        return xT

    with nc.named_scope("ph1_kv"), \
         tc.tile_pool(name="p1w_kv", bufs=1) as wkv, \
         tc.tile_pool(name="p1work_kv", bufs=1) as work, \
         tc.tile_pool(name="p1tp_kv", bufs=2, space="PSUM") as tp1, \
         tc.tile_pool(name="p1psum_kv", bufs=4, space="PSUM") as pps:
        Wk_sb = wkv.tile([128, KC, D], BF16, name="Wk_sb")
        Wv_sb = wkv.tile([128, KC, D], BF16, name="Wv_sb")
        nc.gpsimd.dma_start(out=Wk_sb, in_=Wk.rearrange("(kc p) n -> p kc n", p=128))
        nc.gpsimd.dma_start(out=Wv_sb, in_=Wv.rearrange("(kc p) n -> p kc n", p=128))
        for j in range(NBLK):
            kvT = load_transposed_block(work, tp1, key_value, j, "kv")
            # KT block: out[M=dk chunk m (head), N=s2] += Wk[kc,m].T @ kvT[kc]
            for m in range(H):
                ps = pps.tile([128, BLK], F32, name="ps_k", tag="pps")
                for kc in range(KC):
                    nc.tensor.matmul(
                        ps, lhsT=Wk_sb[:, kc, m * 128:(m + 1) * 128],
                        rhs=kvT[:, kc, :], start=(kc == 0), stop=(kc == KC - 1),
                    )
                nc.scalar.copy(KT_sb[:, m, j * BLK:(j + 1) * BLK], ps)
            # V block rows: out[M=s2 sub, N=hdv] += kvT[kc, sub].T @ Wv[kc]
            for m4 in range(4):
                for n in range(2):
                    ps = pps.tile([128, BLK], F32, name="ps_v", tag="pps")
                    for kc in range(KC):
                        nc.tensor.matmul(
                            ps, lhsT=kvT[:, kc, m4 * 128:(m4 + 1) * 128],
                            rhs=Wv_sb[:, kc, n * BLK:(n + 1) * BLK],
                            start=(kc == 0), stop=(kc == KC - 1),
                        )
                    nc.scalar.copy(
                        V_sb[:, j * 4 + m4, n * BLK:(n + 1) * BLK], ps
                    )

    with nc.named_scope("ph1_q"), \
         tc.tile_pool(name="p1w_q", bufs=1) as wq, \
         tc.tile_pool(name="p1work_q", bufs=1) as work, \
         tc.tile_pool(name="p1tp_q", bufs=2, space="PSUM") as tp1, \
         tc.tile_pool(name="p1psum_q", bufs=4, space="PSUM") as pps:
        Wq_sb = wq.tile([128, KC, D], BF16, name="Wq_sb")
        nc.gpsimd.dma_start(out=Wq_sb, in_=Wq.rearrange("(kc p) n -> p kc n", p=128))
        for j in range(NBLK):
            qT = load_transposed_block(work, tp1, query, j, "q")
            for m in range(H):
                ps = pps.tile([128, BLK], F32, name="ps_q", tag="pps")
                for kc in range(KC):
                    nc.tensor.matmul(
                        ps, lhsT=Wq_sb[:, kc, m * 128:(m + 1) * 128],
                        rhs=qT[:, kc, :], start=(kc == 0), stop=(kc == KC - 1),
                    )
                nc.scalar.copy(QT_sb[:, m, j * BLK:(j + 1) * BLK], ps)

    # ---- phase 2+3: attention + output projection --------------------
    with nc.named_scope("attn"), \
         tc.tile_pool(name="p2", bufs=1) as p2, \
         tc.tile_pool(name="small", bufs=1) as small, \
         tc.tile_pool(name="spsum", bufs=1, space="PSUM") as spsum, \
         tc.tile_pool(name="tpsum", bufs=2, space="PSUM") as tpsum, \
         tc.tile_pool(name="opsum", bufs=1, space="PSUM") as opsum, \
         tc.tile_pool(name="ypsum", bufs=1, space="PSUM") as ypsum:
        for j in range(NBLK):
            OT_sb = p2.tile([128, H, BLK], BF16, name="OT_sb", tag="OT", bufs=2)
            for h in range(H):
                PT_sb = p2.tile([128, SC, BLK], BF16, name="PT_sb", tag="PT", bufs=2)
                for sub in range(4):
                    qcol = j * BLK + sub * 128
                    qtile = QT_sb[:, h, qcol:qcol + 128]
                    P_sb = p2.tile([128, S], BF16, name="P_sb", tag="P", bufs=3)
                    acc = small.tile([128, 1], F32, name="acc", tag="acc", bufs=8)
                    rec = small.tile([128, 1], F32, name="rec", tag="rec", bufs=8)
                    sps = spsum.tile([128, S], F32, name="sps", tag="sps")
                    for n in range(4):
                        nc.tensor.matmul(
                            sps[:, n * BLK:(n + 1) * BLK],
                            lhsT=qtile,
                            rhs=KT_sb[:, h, n * BLK:(n + 1) * BLK],
                            start=True, stop=True,
                        )
                    nc.scalar.activation(
                        P_sb, sps, EXP, scale=SCALE, accum_out=acc,
                    )
                    nc.vector.reciprocal(rec, acc)
                    nc.vector.tensor_scalar_mul(P_sb, P_sb, rec)
                    for g in range(2):
                        pe_transpose8(
                            tpsum,
                            PT_sb[:, g * 8:(g + 1) * 8, sub * 128:(sub + 1) * 128],
                            P_sb, [(g * 8 + i) * 128 for i in range(8)],
                            copy_engine=g % 2,
                        )
                ops = opsum.tile([128, BLK], F32, name="ops", tag="ops")
                for c in range(SC):
                    nc.tensor.matmul(
                        ops, lhsT=V_sb[:, c, h * 128:(h + 1) * 128],
                        rhs=PT_sb[:, c, :], start=(c == 0), stop=(c == SC - 1),
                    )
                nc.vector.tensor_copy(OT_sb[:, h, :], ops)
            # output projection for block j
            for m in range(4):
                for n in range(2):
                    yps = ypsum.tile([128, BLK], F32, name="yps", tag="yps")
                    nc.tensor.matmul(
                        yps, lhsT=ones_sb, rhs=bo_sb[:, n * BLK:(n + 1) * BLK],
                        start=True, stop=False,
                    )
                    for h in range(H):
                        nc.tensor.matmul(
                            yps, lhsT=OT_sb[:, h, m * 128:(m + 1) * 128],
                            rhs=Wo_sb[:, h, n * BLK:(n + 1) * BLK],
                            start=False, stop=(h == H - 1),
                        )
                    y_sb = p2.tile([128, BLK], F32, name="y_sb", tag="y", bufs=3)
                    nc.scalar.copy(y_sb, yps)
                    r0 = j * BLK + m * 128
                    nc.sync.dma_start(
                        out=out[r0:r0 + 128, n * BLK:(n + 1) * BLK], in_=y_sb
                    )
    persist.release()


_CACHE = {}


def _build():
    if "nc" in _CACHE:
        return _CACHE["nc"]
    nc = bacc.Bacc(
        "TRN2", target_bir_lowering=False, debug=False,
        enable_asserts=False, num_devices=B,
    )
    aps = {
        "query": nc.dram_tensor("query", [S, D], F32, kind="ExternalInput").ap(),
        "key_value": nc.dram_tensor("key_value", [S, D], F32, kind="ExternalInput").ap(),
        "Wq": nc.dram_tensor("Wq", [D, H * DK], F32, kind="ExternalInput").ap(),
        "Wk": nc.dram_tensor("Wk", [D, H * DK], F32, kind="ExternalInput").ap(),
        "Wv": nc.dram_tensor("Wv", [D, H * DV], F32, kind="ExternalInput").ap(),
        "Wo": nc.dram_tensor("Wo", [H * DV, D], F32, kind="ExternalInput").ap(),
        "bo": nc.dram_tensor("bo", [1, D], F32, kind="ExternalInput").ap(),
        "out": nc.dram_tensor("out", [S, D], F32, kind="ExternalOutput").ap(),
    }
    with tile.TileContext(nc) as tc:
        _emit(tc, aps)
    nc.compile()
    _CACHE["nc"] = nc
    return nc


LAST_RESULT = None


def kernel(query, key_value, Wq, Wk, Wv, Wo, bo):
    global LAST_RESULT
    nc = _build()
    query = np.ascontiguousarray(np.asarray(query, dtype=np.float32))
    key_value = np.ascontiguousarray(np.asarray(key_value, dtype=np.float32))
    shared = {
        "Wq": np.ascontiguousarray(np.asarray(Wq, dtype=np.float32)),
        "Wk": np.ascontiguousarray(np.asarray(Wk, dtype=np.float32)),
        "Wv": np.ascontiguousarray(np.asarray(Wv, dtype=np.float32)),
        "Wo": np.ascontiguousarray(np.asarray(Wo, dtype=np.float32)),
        "bo": np.ascontiguousarray(np.asarray(bo, dtype=np.float32)).reshape(1, D),
    }
    in_maps = [
        {"query": query[i], "key_value": key_value[i], **shared} for i in range(B)
    ]
    res = run_bass_kernel_spmd(
        nc, in_maps, core_ids=list(range(B)),
        trace=bool(int(os.environ.get("KERNEL_TRACE", "0"))),
    )
    LAST_RESULT = res
    return np.stack([r["out"] for r in res.results]).astype(np.float32)


if __name__ == "__main__":
    rng = np.random.default_rng(0)
    inputs = {
        "query": rng.standard_normal((B, S, D), dtype=np.float32),
        "key_value": rng.standard_normal((B, S, D), dtype=np.float32),
        "Wq": (rng.random((D, H * DK), dtype=np.float32) - 0.5) / 16.0,
        "Wk": (rng.random((D, H * DK), dtype=np.float32) - 0.5) / 16.0,
        "Wv": (rng.random((D, H * DV), dtype=np.float32) - 0.5) / 16.0,
        "Wo": (rng.random((H * DV, D), dtype=np.float32) - 0.5) / 16.0,
        "bo": (rng.random(D, dtype=np.float32) - 0.5) / 16.0,
    }
    y = kernel(**inputs)
    print("kernel out", y.shape, y.dtype, float(np.abs(y).max()))


# revision 19
# speedup vs baseline: 6124.0893x; 1.0005x over previous
"""Cross-attention Trainium2 kernel (Bass/Tile), data-parallel over batch.

B=8 batch elements -> 8 NeuronCores, one batch element per core.
Per core: y = softmax(q Wq (kv Wk)^T / sqrt(dk)) (kv Wv) Wo + bo
with S1=S2=2048, D=1024, H=8, DK=DV=128.

Layout strategy (everything bf16 on the PE, fp32 softmax stats):
  - inputs are cast fp32->bf16 during the SWDGE load, then DMA-xbar-transposed
    to qT/kvT [D, S] tiles.
  - projections produce QT,KT  [H*DK, S] (head-major partition chunks) and
    V [S2, H*DV] (natural), all bf16 in SBUF.
  - scores S = QT_h^T KT_h computed per 128-row q-block into PSUM, exp on ACT
    with fused accum_out row-sums (no max subtraction: |s*scale| < ~3),
    normalize P on DVE, DMA-xbar-transpose P -> PT, then O^T = sum_c V_c^T PT_c
    so the output projection can consume O^T directly with Wo natural.
  - bias bo is folded in as a K=1 ones x bo matmul that opens each output
    accumulation group.
"""

import os

import numpy as np

import concourse.bass as bass
import concourse.mybir as mybir
import concourse.tile as tile
from concourse import bacc
from concourse.bass_utils import run_bass_kernel_spmd
from concourse.masks import make_identity

B = 8
S = 2048  # S1 == S2
D = 1024  # D1 == D2
H = 8
DK = DV = 128
KC = D // 128  # contraction chunks
SC = S // 128  # sequence chunks of 128
BLK = 512
NBLK = S // BLK
SCALE = 1.0 / float(np.sqrt(DK))

F32 = mybir.dt.float32
BF16 = mybir.dt.bfloat16
EXP = mybir.ActivationFunctionType.Exp


def _emit(tc, aps):
    nc = tc.nc
    query, key_value, Wq, Wk, Wv, Wo, bo, out = (
        aps["query"], aps["key_value"], aps["Wq"], aps["Wk"], aps["Wv"],
        aps["Wo"], aps["bo"], aps["out"],
    )

    persist = tc.alloc_tile_pool(name="persist", bufs=1)
    QT_sb = persist.tile([128, H, S], BF16, name="QT_sb")
    KT_sb = persist.tile([128, H, S], BF16, name="KT_sb")
    V_sb = persist.tile([128, SC, H * DV], BF16, name="V_sb")
    Wo_sb = persist.tile([128, KC, D], BF16, name="Wo_sb")
    bo_sb = persist.tile([1, D], BF16, name="bo_sb")
    ones_sb = persist.tile([1, 128], BF16, name="ones_sb")

    ident = persist.tile([128, 128], BF16, name="ident")
    make_identity(nc, ident)
    nc.vector.memset(ones_sb, 1.0)
    nc.gpsimd.dma_start(out=bo_sb, in_=bo)  # casts f32 -> bf16

    def load_weight(dst, src):
        # split the cast-DMA per 128-row chunk so dependent matmuls can
        # start as soon as their contraction chunk lands
        srcv = src.rearrange("(kc p) n -> p kc n", p=128)
        for kc in range(KC):
            nc.gpsimd.dma_start(out=dst[:, kc, :], in_=srcv[:, kc, :])

    load_weight(Wo_sb, Wo)

    def pe_transpose8(tpool, dst8, src, cols, copy_engine):
        """Transpose eight [128,128] bf16 tiles of src (cols slice list) through
        one PSUM bank and copy into dst8 [128, 8, 128]."""
        tp = tpool.tile([128, 1024], BF16, name="tp", tag="tp")
        for i, c0 in enumerate(cols):
            nc.tensor.transpose(
                tp[:, i * 128:(i + 1) * 128], src[:, c0:c0 + 128], ident
            )
        srcv = tp.rearrange("p (c f) -> p c f", c=8)
        if copy_engine == 0:
            nc.vector.tensor_copy(dst8, srcv)
        else:
            nc.scalar.copy(dst8, srcv)

    # ---- phase 1: projections ----------------------------------------
    def load_transposed_block(work, tpool, src_ap, j, tag):
        """Load 512 rows of src [S, D] f32, return xT block [128, KC, 512] bf16."""
        xT = work.tile([128, KC, BLK], BF16, name=f"{tag}T", tag=f"{tag}T", bufs=2)
        for c4 in range(4):
            c = j * 4 + c4
            row = work.tile([128, D], BF16, name=f"{tag}row", tag="row", bufs=3)
            nc.gpsimd.dma_start(out=row, in_=src_ap[c * 128:(c + 1) * 128, :])
            pe_transpose8(
                tpool,
                xT[:, :, c4 * 128:(c4 + 1) * 128],
                row, [kc * 128 for kc in range(KC)],
                copy_engine=c4 % 2,
            )
        return xT

    with nc.named_scope("ph1_kv"), \
         tc.tile_pool(name="p1w_kv", bufs=1) as wkv, \
         tc.tile_pool(name="p1work_kv", bufs=1) as work, \
         tc.tile_pool(name="p1tp_kv", bufs=2, space="PSUM") as tp1, \
         tc.tile_pool(name="p1psum_kv", bufs=4, space="PSUM") as pps:
        Wk_sb = wkv.tile([128, KC, D], BF16, name="Wk_sb")
        Wv_sb = wkv.tile([128, KC, D], BF16, name="Wv_sb")
        load_weight(Wk_sb, Wk)
        load_weight(Wv_sb, Wv)
        for j in range(NBLK):
            kvT = load_transposed_block(work, tp1, key_value, j, "kv")
            # KT block: out[M=dk chunk m (head), N=s2] += Wk[kc,m].T @ kvT[kc]
            for m in range(H):
                ps = pps.tile([128, BLK], F32, name="ps_k", tag="pps")
                for kc in range(KC):
                    nc.tensor.matmul(
                        ps, lhsT=Wk_sb[:, kc, m * 128:(m + 1) * 128],
                        rhs=kvT[:, kc, :], start=(kc == 0), stop=(kc == KC - 1),
                    )
                nc.scalar.copy(KT_sb[:, m, j * BLK:(j + 1) * BLK], ps)
            # V block rows: out[M=s2 sub, N=hdv] += kvT[kc, sub].T @ Wv[kc]
            for m4 in range(4):
                for n in range(2):
                    ps = pps.tile([128, BLK], F32, name="ps_v", tag="pps")
                    for kc in range(KC):
                        nc.tensor.matmul(
                            ps, lhsT=kvT[:, kc, m4 * 128:(m4 + 1) * 128],
                            rhs=Wv_sb[:, kc, n * BLK:(n + 1) * BLK],
                            start=(kc == 0), stop=(kc == KC - 1),
                        )
                    nc.scalar.copy(
                        V_sb[:, j * 4 + m4, n * BLK:(n + 1) * BLK], ps
                    )

    with nc.named_scope("ph1_q"), \
         tc.tile_pool(name="p1w_q", bufs=1) as wq, \
         tc.tile_pool(name="p1work_q", bufs=1) as work, \
         tc.tile_pool(name="p1tp_q", bufs=2, space="PSUM") as tp1, \
         tc.tile_pool(name="p1psum_q", bufs=4, space="PSUM") as pps:
        Wq_sb = wq.tile([128, KC, D], BF16, name="Wq_sb")
        load_weight(Wq_sb, Wq)
        for j in range(NBLK):
            qT = load_transposed_block(work, tp1, query, j, "q")
            for m in range(H):
                ps = pps.tile([128, BLK], F32, name="ps_q", tag="pps")
                for kc in range(KC):
                    nc.tensor.matmul(
                        ps, lhsT=Wq_sb[:, kc, m * 128:(m + 1) * 128],
                        rhs=qT[:, kc, :], start=(kc == 0), stop=(kc == KC - 1),
                    )
                nc.scalar.copy(QT_sb[:, m, j * BLK:(j + 1) * BLK], ps)

    # ---- phase 2+3: attention + output projection --------------------
    with nc.named_scope("attn"), \
         tc.tile_pool(name="p2", bufs=1) as p2, \
         tc.tile_pool(name="small", bufs=1) as small, \
         tc.tile_pool(name="spsum", bufs=2, space="PSUM") as spsum, \
         tc.tile_pool(name="tpsum", bufs=2, space="PSUM") as tpsum, \
         tc.tile_pool(name="opsum", bufs=1, space="PSUM") as opsum, \
         tc.tile_pool(name="ypsum", bufs=1, space="PSUM") as ypsum:
        for j in range(NBLK):
            OT_sb = p2.tile([128, H, BLK], BF16, name="OT_sb", tag="OT", bufs=2)
            for h in range(H):
                PT_sb = p2.tile([128, SC, BLK], BF16, name="PT_sb", tag="PT", bufs=2)
                for sub in range(4):
                    qcol = j * BLK + sub * 128
                    qtile = QT_sb[:, h, qcol:qcol + 128]
                    P_sb = p2.tile([128, S], BF16, name="P_sb", tag="P", bufs=3)
                    ssum = small.tile([128, 1], F32, name="ssum", tag="ssum", bufs=8)
                    rec = small.tile([128, 1], F32, name="rec", tag="rec", bufs=8)
                    accs = []
                    for half in range(2):
                        sps = spsum.tile([128, 1024], F32, name="sps", tag="sps")
                        for n in range(2):
                            c0 = half * 1024 + n * BLK
                            nc.tensor.matmul(
                                sps[:, n * BLK:(n + 1) * BLK],
                                lhsT=qtile,
                                rhs=KT_sb[:, h, c0:c0 + BLK],
                                start=True, stop=True,
                            )
                        acc = small.tile([128, 1], F32, name="acc",
                                         tag=f"acc{half}", bufs=4)
                        nc.scalar.activation(
                            P_sb[:, half * 1024:(half + 1) * 1024], sps, EXP,
                            scale=SCALE, accum_out=acc,
                        )
                        accs.append(acc)
                    nc.vector.tensor_add(ssum, accs[0], accs[1])
                    nc.vector.reciprocal(rec, ssum)
                    for g in range(2):
                        nc.vector.tensor_scalar_mul(
                            P_sb[:, g * 1024:(g + 1) * 1024],
                            P_sb[:, g * 1024:(g + 1) * 1024], rec,
                        )
                        pe_transpose8(
                            tpsum,
                            PT_sb[:, g * 8:(g + 1) * 8, sub * 128:(sub + 1) * 128],
                            P_sb, [(g * 8 + i) * 128 for i in range(8)],
                            copy_engine=g % 2,
                        )
                ops = opsum.tile([128, BLK], F32, name="ops", tag="ops")
                for c in range(SC):
                    nc.tensor.matmul(
                        ops, lhsT=V_sb[:, c, h * 128:(h + 1) * 128],
                        rhs=PT_sb[:, c, :], start=(c == 0), stop=(c == SC - 1),
                    )
                nc.vector.tensor_copy(OT_sb[:, h, :], ops)
            # output projection for block j
            for m in range(4):
                for n in range(2):
                    yps = ypsum.tile([128, BLK], F32, name="yps", tag="yps")
                    nc.tensor.matmul(
                        yps, lhsT=ones_sb, rhs=bo_sb[:, n * BLK:(n + 1) * BLK],
                        start=True, stop=False,
                    )
                    for h in range(H):
                        nc.tensor.matmul(
                            yps, lhsT=OT_sb[:, h, m * 128:(m + 1) * 128],
                            rhs=Wo_sb[:, h, n * BLK:(n + 1) * BLK],
                            start=False, stop=(h == H - 1),
                        )
                    y_sb = p2.tile([128, BLK], F32, name="y_sb", tag="y", bufs=3)
                    nc.scalar.copy(y_sb, yps)
                    r0 = j * BLK + m * 128
                    nc.sync.dma_start(
                        out=out[r0:r0 + 128, n * BLK:(n + 1) * BLK], in_=y_sb
                    )
    persist.release()


_CACHE = {}


def _build():
    if "nc" in _CACHE:
        return _CACHE["nc"]
    nc = bacc.Bacc(
        "TRN2", target_bir_lowering=False, debug=False,
        enable_asserts=False, num_devices=B,
    )
    aps = {
        "query": nc.dram_tensor("query", [S, D], F32, kind="ExternalInput").ap(),
        "key_value": nc.dram_tensor("key_value", [S, D], F32, kind="ExternalInput").ap(),
        "Wq": nc.dram_tensor("Wq", [D, H * DK], F32, kind="ExternalInput").ap(),
        "Wk": nc.dram_tensor("Wk", [D, H * DK], F32, kind="ExternalInput").ap(),
        "Wv": nc.dram_tensor("Wv", [D, H * DV], F32, kind="ExternalInput").ap(),
        "Wo": nc.dram_tensor("Wo", [H * DV, D], F32, kind="ExternalInput").ap(),
        "bo": nc.dram_tensor("bo", [1, D], F32, kind="ExternalInput").ap(),
        "out": nc.dram_tensor("out", [S, D], F32, kind="ExternalOutput").ap(),
    }
    with tile.TileContext(nc) as tc:
        _emit(tc, aps)
    nc.compile()
    _CACHE["nc"] = nc
    return nc


LAST_RESULT = None


def kernel(query, key_value, Wq, Wk, Wv, Wo, bo):
    global LAST_RESULT
    nc = _build()
    query = np.ascontiguousarray(np.asarray(query, dtype=np.float32))
    key_value = np.ascontiguousarray(np.asarray(key_value, dtype=np.float32))
    shared = {
        "Wq": np.ascontiguousarray(np.asarray(Wq, dtype=np.float32)),
        "Wk": np.ascontiguousarray(np.asarray(Wk, dtype=np.float32)),
        "Wv": np.ascontiguousarray(np.asarray(Wv, dtype=np.float32)),
        "Wo": np.ascontiguousarray(np.asarray(Wo, dtype=np.float32)),
        "bo": np.ascontiguousarray(np.asarray(bo, dtype=np.float32)).reshape(1, D),
    }
    in_maps = [
        {"query": query[i], "key_value": key_value[i], **shared} for i in range(B)
    ]
    res = run_bass_kernel_spmd(
        nc, in_maps, core_ids=list(range(B)),
        trace=bool(int(os.environ.get("KERNEL_TRACE", "0"))),
    )
    LAST_RESULT = res
    return np.stack([r["out"] for r in res.results]).astype(np.float32)


if __name__ == "__main__":
    rng = np.random.default_rng(0)
    inputs = {
        "query": rng.standard_normal((B, S, D), dtype=np.float32),
        "key_value": rng.standard_normal((B, S, D), dtype=np.float32),
        "Wq": (rng.random((D, H * DK), dtype=np.float32) - 0.5) / 16.0,
        "Wk": (rng.random((D, H * DK), dtype=np.float32) - 0.5) / 16.0,
        "Wv": (rng.random((D, H * DV), dtype=np.float32) - 0.5) / 16.0,
        "Wo": (rng.random((H * DV, D), dtype=np.float32) - 0.5) / 16.0,
        "bo": (rng.random(D, dtype=np.float32) - 0.5) / 16.0,
    }
    y = kernel(**inputs)
    print("kernel out", y.shape, y.dtype, float(np.abs(y).max()))


# revision 24
# speedup vs baseline: 6363.5822x; 1.0391x over previous
"""Cross-attention Trainium2 kernel (Bass/Tile), data-parallel over batch.

B=8 batch elements -> 8 NeuronCores, one batch element per core.
Per core: y = softmax(q Wq (kv Wk)^T / sqrt(dk)) (kv Wv) Wo + bo
with S1=S2=2048, D=1024, H=8, DK=DV=128.

Layout strategy (everything bf16 on the PE, fp32 softmax stats):
  - inputs are cast fp32->bf16 during the SWDGE load, then DMA-xbar-transposed
    to qT/kvT [D, S] tiles.
  - projections produce QT,KT  [H*DK, S] (head-major partition chunks) and
    V [S2, H*DV] (natural), all bf16 in SBUF.
  - scores S = QT_h^T KT_h computed per 128-row q-block into PSUM, exp on ACT
    with fused accum_out row-sums (no max subtraction: |s*scale| < ~3),
    normalize P on DVE, DMA-xbar-transpose P -> PT, then O^T = sum_c V_c^T PT_c
    so the output projection can consume O^T directly with Wo natural.
  - bias bo is folded in as a K=1 ones x bo matmul that opens each output
    accumulation group.
"""

import os

import numpy as np

import concourse.bass as bass
import concourse.mybir as mybir
import concourse.tile as tile
from concourse import bacc
from concourse.bass_utils import run_bass_kernel_spmd
from concourse.masks import make_identity

B = 8
S = 2048  # S1 == S2
D = 1024  # D1 == D2
H = 8
DK = DV = 128
KC = D // 128  # contraction chunks
SC = S // 128  # sequence chunks of 128
BLK = 512
NBLK = S // BLK
SCALE = 1.0 / float(np.sqrt(DK))

F32 = mybir.dt.float32
BF16 = mybir.dt.bfloat16
EXP = mybir.ActivationFunctionType.Exp


def _emit(tc, aps):
    nc = tc.nc
    query, key_value, Wq, Wk, Wv, Wo, bo, out = (
        aps["query"], aps["key_value"], aps["Wq"], aps["Wk"], aps["Wv"],
        aps["Wo"], aps["bo"], aps["out"],
    )

    persist = tc.alloc_tile_pool(name="persist", bufs=1)
    QT_sb = persist.tile([128, H, S], BF16, name="QT_sb")
    KT_sb = persist.tile([128, H, S], BF16, name="KT_sb")
    V_sb = persist.tile([128, SC, H * DV], BF16, name="V_sb")
    Wo_sb = persist.tile([128, KC, D], BF16, name="Wo_sb")
    bo_sb = persist.tile([1, D], BF16, name="bo_sb")
    ones_sb = persist.tile([1, 128], BF16, name="ones_sb")
    onec_sb = persist.tile([128, 1], BF16, name="onec_sb")

    ident = persist.tile([128, 128], BF16, name="ident")
    make_identity(nc, ident)
    nc.vector.memset(ones_sb, 1.0)
    nc.vector.memset(onec_sb, 1.0)
    nc.gpsimd.dma_start(out=bo_sb, in_=bo)  # casts f32 -> bf16

    def load_weight(dst, src):
        # split the cast-DMA per 128-row chunk so dependent matmuls can
        # start as soon as their contraction chunk lands
        srcv = src.rearrange("(kc p) n -> p kc n", p=128)
        for kc in range(KC):
            nc.gpsimd.dma_start(out=dst[:, kc, :], in_=srcv[:, kc, :])

    load_weight(Wo_sb, Wo)

    def pe_transpose8(tpool, dst8, src, cols, copy_engine):
        """Transpose eight [128,128] bf16 tiles of src (cols slice list) through
        one PSUM bank and copy into dst8 [128, 8, 128]."""
        tp = tpool.tile([128, 1024], BF16, name="tp", tag="tp")
        for i, c0 in enumerate(cols):
            nc.tensor.transpose(
                tp[:, i * 128:(i + 1) * 128], src[:, c0:c0 + 128], ident
            )
        srcv = tp.rearrange("p (c f) -> p c f", c=8)
        if copy_engine == 0:
            nc.vector.tensor_copy(dst8, srcv)
        else:
            nc.scalar.copy(dst8, srcv)

    # ---- phase 1: projections ----------------------------------------
    def load_transposed_block(work, tpool, src_ap, j, tag):
        """Load 512 rows of src [S, D] f32, return xT block [128, KC, 512] bf16."""
        xT = work.tile([128, KC, BLK], BF16, name=f"{tag}T", tag=f"{tag}T", bufs=2)
        for c4 in range(4):
            c = j * 4 + c4
            row = work.tile([128, D], BF16, name=f"{tag}row", tag="row", bufs=3)
            nc.gpsimd.dma_start(out=row, in_=src_ap[c * 128:(c + 1) * 128, :])
            pe_transpose8(
                tpool,
                xT[:, :, c4 * 128:(c4 + 1) * 128],
                row, [kc * 128 for kc in range(KC)],
                copy_engine=c4 % 2,
            )
        return xT

    with nc.named_scope("ph1_kv"), \
         tc.tile_pool(name="p1w_kv", bufs=1) as wkv, \
         tc.tile_pool(name="p1work_kv", bufs=1) as work, \
         tc.tile_pool(name="p1tp_kv", bufs=2, space="PSUM") as tp1, \
         tc.tile_pool(name="p1psum_kv", bufs=4, space="PSUM") as pps:
        Wk_sb = wkv.tile([128, KC, D], BF16, name="Wk_sb")
        Wv_sb = wkv.tile([128, KC, D], BF16, name="Wv_sb")
        load_weight(Wk_sb, Wk)
        load_weight(Wv_sb, Wv)
        for j in range(NBLK):
            kvT = load_transposed_block(work, tp1, key_value, j, "kv")
            # KT block: out[M=dk chunk m (head), N=s2] += Wk[kc,m].T @ kvT[kc]
            for m in range(H):
                ps = pps.tile([128, BLK], F32, name="ps_k", tag="pps")
                for kc in range(KC):
                    nc.tensor.matmul(
                        ps, lhsT=Wk_sb[:, kc, m * 128:(m + 1) * 128],
                        rhs=kvT[:, kc, :], start=(kc == 0), stop=(kc == KC - 1),
                    )
                nc.scalar.copy(KT_sb[:, m, j * BLK:(j + 1) * BLK], ps)
            # V block rows: out[M=s2 sub, N=hdv] += kvT[kc, sub].T @ Wv[kc]
            for m4 in range(4):
                for n in range(2):
                    ps = pps.tile([128, BLK], F32, name="ps_v", tag="pps")
                    for kc in range(KC):
                        nc.tensor.matmul(
                            ps, lhsT=kvT[:, kc, m4 * 128:(m4 + 1) * 128],
                            rhs=Wv_sb[:, kc, n * BLK:(n + 1) * BLK],
                            start=(kc == 0), stop=(kc == KC - 1),
                        )
                    nc.scalar.copy(
                        V_sb[:, j * 4 + m4, n * BLK:(n + 1) * BLK], ps
                    )

    with nc.named_scope("ph1_q"), \
         tc.tile_pool(name="p1w_q", bufs=1) as wq, \
         tc.tile_pool(name="p1work_q", bufs=1) as work, \
         tc.tile_pool(name="p1tp_q", bufs=2, space="PSUM") as tp1, \
         tc.tile_pool(name="p1psum_q", bufs=4, space="PSUM") as pps:
        Wq_sb = wq.tile([128, KC, D], BF16, name="Wq_sb")
        load_weight(Wq_sb, Wq)
        for j in range(NBLK):
            qT = load_transposed_block(work, tp1, query, j, "q")
            for m in range(H):
                ps = pps.tile([128, BLK], F32, name="ps_q", tag="pps")
                for kc in range(KC):
                    nc.tensor.matmul(
                        ps, lhsT=Wq_sb[:, kc, m * 128:(m + 1) * 128],
                        rhs=qT[:, kc, :], start=(kc == 0), stop=(kc == KC - 1),
                    )
                nc.scalar.copy(QT_sb[:, m, j * BLK:(j + 1) * BLK], ps)

    # ---- phase 2+3: attention + output projection --------------------
    with nc.named_scope("attn"), \
         tc.tile_pool(name="p2", bufs=1) as p2, \
         tc.tile_pool(name="small", bufs=1) as small, \
         tc.tile_pool(name="spsum", bufs=1, space="PSUM") as spsum, \
         tc.tile_pool(name="supsum", bufs=1, space="PSUM") as supsum, \
         tc.tile_pool(name="opsum", bufs=2, space="PSUM") as opsum, \
         tc.tile_pool(name="ypsum", bufs=1, space="PSUM") as ypsum, \
         tc.tile_pool(name="dram", bufs=4, space="DRAM") as dpool:
        for j in range(NBLK):
            OT_sb = p2.tile([128, H, BLK], BF16, name="OT_sb", tag="OT", bufs=2)
            jcols = slice(j * BLK, (j + 1) * BLK)
            for h in range(H):
                # scores transposed: ST[c][s2_local, s1] = K_h^T q, exp -> PT
                PT_sb = p2.tile([128, SC, BLK], BF16, name="PT_sb", tag="PT", bufs=2)
                qblk = QT_sb[:, h, jcols]
                for g in range(4):
                    sps = spsum.tile([128, 4 * BLK], F32, name="sps", tag="sps")
                    for i in range(4):
                        c = 4 * g + i
                        nc.tensor.matmul(
                            sps[:, i * BLK:(i + 1) * BLK],
                            lhsT=KT_sb[:, h, c * 128:(c + 1) * 128],
                            rhs=qblk, start=True, stop=True,
                        )
                    nc.scalar.activation(
                        PT_sb[:, 4 * g:4 * (g + 1), :],
                        sps.rearrange("p (c n) -> p c n", c=4),
                        EXP, scale=SCALE,
                    )
                # row sums (over s2) via ones-column matmul; bcast reciprocal
                sus = supsum.tile([1, BLK], F32, name="sus", tag="sus")
                for c in range(SC):
                    nc.tensor.matmul(
                        sus, lhsT=onec_sb, rhs=PT_sb[:, c, :],
                        start=(c == 0), stop=(c == SC - 1),
                    )
                rec_row = small.tile([1, BLK], F32, name="rec_row", tag="rec",
                                     bufs=4)
                nc.vector.reciprocal(rec_row, sus)
                rec_d = dpool.tile([1, BLK], F32, name="rec_d", tag="rec_d")
                nc.sync.dma_start(out=rec_d, in_=rec_row)
                bc_sb = small.tile([128, BLK], F32, name="bc_sb", tag="bc",
                                   bufs=2)
                rec_bcast = bass.AP(
                    tensor=rec_d.tensor, offset=rec_d.offset,
                    ap=[[0, 128]] + list(rec_d.ap[1:]),
                )
                nc.gpsimd.dma_start(out=bc_sb, in_=rec_bcast)
                ops = opsum.tile([128, BLK], F32, name="ops", tag="ops")
                for c in range(SC):
                    nc.tensor.matmul(
                        ops, lhsT=V_sb[:, c, h * 128:(h + 1) * 128],
                        rhs=PT_sb[:, c, :], start=(c == 0), stop=(c == SC - 1),
                    )
                nc.vector.tensor_mul(OT_sb[:, h, :], ops, bc_sb)
            # output projection for block j
            for m in range(4):
                for n in range(2):
                    yps = ypsum.tile([128, BLK], F32, name="yps", tag="yps")
                    nc.tensor.matmul(
                        yps, lhsT=ones_sb, rhs=bo_sb[:, n * BLK:(n + 1) * BLK],
                        start=True, stop=False,
                    )
                    for h in range(H):
                        nc.tensor.matmul(
                            yps, lhsT=OT_sb[:, h, m * 128:(m + 1) * 128],
                            rhs=Wo_sb[:, h, n * BLK:(n + 1) * BLK],
                            start=False, stop=(h == H - 1),
                        )
                    y_sb = p2.tile([128, BLK], F32, name="y_sb", tag="y", bufs=3)
                    nc.scalar.copy(y_sb, yps)
                    r0 = j * BLK + m * 128
                    nc.sync.dma_start(
                        out=out[r0:r0 + 128, n * BLK:(n + 1) * BLK], in_=y_sb
                    )
    persist.release()


_CACHE = {}


def _build():
    if "nc" in _CACHE:
        return _CACHE["nc"]
    nc = bacc.Bacc(
        "TRN2", target_bir_lowering=False, debug=False,
        enable_asserts=False, num_devices=B,
    )
    aps = {
        "query": nc.dram_tensor("query", [S, D], F32, kind="ExternalInput").ap(),
        "key_value": nc.dram_tensor("key_value", [S, D], F32, kind="ExternalInput").ap(),
        "Wq": nc.dram_tensor("Wq", [D, H * DK], F32, kind="ExternalInput").ap(),
        "Wk": nc.dram_tensor("Wk", [D, H * DK], F32, kind="ExternalInput").ap(),
        "Wv": nc.dram_tensor("Wv", [D, H * DV], F32, kind="ExternalInput").ap(),
        "Wo": nc.dram_tensor("Wo", [H * DV, D], F32, kind="ExternalInput").ap(),
        "bo": nc.dram_tensor("bo", [1, D], F32, kind="ExternalInput").ap(),
        "out": nc.dram_tensor("out", [S, D], F32, kind="ExternalOutput").ap(),
    }
    with tile.TileContext(nc) as tc:
        _emit(tc, aps)
    nc.compile()
    _CACHE["nc"] = nc
    return nc


LAST_RESULT = None


def kernel(query, key_value, Wq, Wk, Wv, Wo, bo):
    global LAST_RESULT
    nc = _build()
    query = np.ascontiguousarray(np.asarray(query, dtype=np.float32))
    key_value = np.ascontiguousarray(np.asarray(key_value, dtype=np.float32))
    shared = {
        "Wq": np.ascontiguousarray(np.asarray(Wq, dtype=np.float32)),
        "Wk": np.ascontiguousarray(np.asarray(Wk, dtype=np.float32)),
        "Wv": np.ascontiguousarray(np.asarray(Wv, dtype=np.float32)),
        "Wo": np.ascontiguousarray(np.asarray(Wo, dtype=np.float32)),
        "bo": np.ascontiguousarray(np.asarray(bo, dtype=np.float32)).reshape(1, D),
    }
    in_maps = [
        {"query": query[i], "key_value": key_value[i], **shared} for i in range(B)
    ]
    res = run_bass_kernel_spmd(
        nc, in_maps, core_ids=list(range(B)),
        trace=bool(int(os.environ.get("KERNEL_TRACE", "0"))),
    )
    LAST_RESULT = res
    return np.stack([r["out"] for r in res.results]).astype(np.float32)


if __name__ == "__main__":
    rng = np.random.default_rng(0)
    inputs = {
        "query": rng.standard_normal((B, S, D), dtype=np.float32),
        "key_value": rng.standard_normal((B, S, D), dtype=np.float32),
        "Wq": (rng.random((D, H * DK), dtype=np.float32) - 0.5) / 16.0,
        "Wk": (rng.random((D, H * DK), dtype=np.float32) - 0.5) / 16.0,
        "Wv": (rng.random((D, H * DV), dtype=np.float32) - 0.5) / 16.0,
        "Wo": (rng.random((H * DV, D), dtype=np.float32) - 0.5) / 16.0,
        "bo": (rng.random(D, dtype=np.float32) - 0.5) / 16.0,
    }
    y = kernel(**inputs)
    print("kernel out", y.shape, y.dtype, float(np.abs(y).max()))
